# revision 19
# baseline (speedup 1.0000x reference)
"""Trainium2 Bass kernel for the Luong-attention LSTM decoder (nn_Decoder).

8-core strategy (v2, sharded recurrence + interleaved epilogue):
- Host folds Wa into the recurrence: z = [H*, ctx] @ wz + xp with
  wz = [0.5*(Wa_top@Wk_a + lstm_r); Wa_bot@Wk_a], xp precomputed (+bias,
  t=0 correction). State H* = 2*h2 (tanh-only gates, no ACT table
  switches); consumers pre-scaled by 0.5 on host.
- z matmul + gates column-sharded 8-way: core r computes gate columns
  [i_r|f_r|o_r|g_r] (128 each), keeps its c-state slice. AllGather #1
  reassembles transposed H*; AllGather #2 reassembles ctx. Single-DMA
  gathers.
- Attention batch-sharded (8 samples/core): cross-scores via 8 matmuls +
  matmul diagonal-select (SPMD-uniform via per-core one-hot isel).
- Epilogue (attn2 = [H*,ctx]@wa_eff, logits vocab-sharded 4000/core) is
  chunked and pumped into the AG-wait windows of later steps, keeping the
  PE HAM-warm and removing the serial tail. Pair history kept in SBUF.
"""
import sys

sys.path.insert(0, "/opt/trn_rl_repo")

from collections import deque

import numpy as np
import concourse.bass as bass
import concourse.tile as tile
from concourse import bacc, mybir
from concourse.bass_utils import run_bass_kernel_spmd

B, T_IN, T_DEC = 64, 64, 47
V, E, H = 32000, 512, 1024
NC = 8
BL = B // NC
VS = V // NC
KC = H // 128
NPAIR = BL // 2
GS = VS // 8
NPAIRS_T = (T_DEC + 1) // 2
F32 = mybir.dt.float32
F16 = mybir.dt.float16
BF16 = mybir.dt.bfloat16
TANH = mybir.ActivationFunctionType.Tanh
EXP = mybir.ActivationFunctionType.Exp
MULT = mybir.AluOpType.mult
ADD = mybir.AluOpType.add

_CACHE = {}


def _build():
    nc = bacc.Bacc(None, target_bir_lowering=False)

    wz_d = nc.dram_tensor("wz", [16, 128, 512], BF16, kind="ExternalInput")
    xp_d = nc.dram_tensor("xp", [T_DEC, B, 512], BF16, kind="ExternalInput")
    c0_d = nc.dram_tensor("c0", [B, 128], F32, kind="ExternalInput")
    keysk_d = nc.dram_tensor("keysk", [128, KC, BL, T_IN], BF16, kind="ExternalInput")
    mempk_d = nc.dram_tensor("mempk", [128, NPAIR, KC, 128], F16, kind="ExternalInput")
    isel_d = nc.dram_tensor("isel", [B, BL], F32, kind="ExternalInput")
    idf_d = nc.dram_tensor("idf", [64, 64], F32, kind="ExternalInput")
    wa_d = nc.dram_tensor("wa", [16, 128, H], BF16, kind="ExternalInput")
    fcw_d = nc.dram_tensor("fcw", [KC, 128, VS], BF16, kind="ExternalInput")
    fcbr_d = nc.dram_tensor("fcbr", [128, VS], F32, kind="ExternalInput")
    idb_d = nc.dram_tensor("idb", [128, 128], BF16, kind="ExternalInput")
    out_d = nc.dram_tensor("out", [B, T_DEC, VS], F32, kind="ExternalOutput")

    ag1i = [nc.dram_tensor(f"ag1i{t}", [64, 128], BF16) for t in range(T_DEC)]
    ag1o = [
        nc.dram_tensor(f"ag1o{t}", [NC * 64, 128], BF16, addr_space="Shared")
        for t in range(T_DEC)
    ]
    ag2i = [nc.dram_tensor(f"ag2i{t}", [128, 64], BF16) for t in range(T_DEC)]
    ag2o = [
        nc.dram_tensor(f"ag2o{t}", [NC * 128, 64], BF16, addr_space="Shared")
        for t in range(T_DEC)
    ]
    rg = [list(range(NC))]

    with tile.TileContext(nc) as tc:
        with (
            tc.tile_pool(name="one", bufs=1) as one,
            tc.tile_pool(name="work", bufs=2) as work,
            tc.tile_pool(name="gat", bufs=2) as gat,
            tc.tile_pool(name="hist", bufs=3) as hist,
            tc.tile_pool(name="ework", bufs=2) as ework,
            tc.tile_pool(name="zps", bufs=2, space="PSUM") as zps,
            tc.tile_pool(name="zpsB", bufs=1, space="PSUM") as zpsB,
            tc.tile_pool(name="tps", bufs=1, space="PSUM") as tps,
            tc.tile_pool(name="aps", bufs=1, space="PSUM") as aps,
            tc.tile_pool(name="eps", bufs=1, space="PSUM") as eps,
            tc.tile_pool(name="fps", bufs=1, space="PSUM") as fps,
        ):
            # ---------------- resident tiles ----------------
            wz = one.tile([128, 16, 512], BF16)
            nc.sync.dma_start(wz[:], wz_d.rearrange("k p n -> p k n"))
            keysK = one.tile([128, KC, BL, T_IN], BF16)
            nc.sync.dma_start(keysK[:], keysk_d[:])
            memPK = one.tile([128, NPAIR, KC, 128], F16)
            nc.sync.dma_start(memPK[:], mempk_d[:])
            isel = one.tile([B, BL], F32, tag="isel")
            nc.sync.dma_start(isel[:], isel_d[:])
            idf = one.tile([64, 64], F32, tag="idf")
            nc.sync.dma_start(idf[:], idf_d[:])
            cst = one.tile([B, 128], F32, tag="cst")
            nc.sync.dma_start(cst[:], c0_d[:])
            wa_sb = one.tile([128, 16, H], BF16)
            nc.sync.dma_start(wa_sb[:], wa_d.rearrange("k p h -> p k h"))
            fcw_sb = one.tile([128, KC, VS], BF16)
            nc.sync.dma_start(fcw_sb[:], fcw_d.rearrange("k p v -> p k v"))
            identb = one.tile([128, 128], BF16)
            nc.sync.dma_start(identb[:], idb_d[:])
            fcbR = one.tile([128, VS], F32)
            nc.sync.dma_start(fcbR[:], fcbr_d[:])

            h2T = one.tile([128, KC, 64], BF16, tag="h2T")
            nc.vector.memset(h2T[:], 0.0)
            ctxT = one.tile([128, NC, KC, BL], BF16, tag="ctxT")
            nc.vector.memset(ctxT[:], 0.0)
            ctxTz = one.tile([128, KC, NC, BL], BF16, tag="ctxTz")
            nc.vector.memset(ctxTz[:], 0.0)
            alignZ = one.tile([128, BL], F16, tag="alignZ")
            nc.vector.memset(alignZ[:], 0.0)
            ones64 = one.tile([B, 1], F32, tag="ones64")
            nc.vector.memset(ones64[:], 1.0)
            onesr = one.tile([1, B], F32, tag="onesr")
            nc.vector.memset(onesr[:], 1.0)

            # ------------- epilogue chunk machinery -------------
            epiq = deque()

            def pump(budget_us):
                spent = 0.0
                while epiq and spent < budget_us:
                    cost, go = epiq.popleft()
                    go()
                    spent += cost

            def make_pair_chunks(t0, nsteps, h2p, ctxp):
                M = 64 * nsteps
                a2sb = ework.tile([128, H], BF16, tag="a2sb",
                                  name=f"a2sb{t0}")
                a2t = ework.tile([128, KC, 128], BF16, tag="a2t",
                                 name=f"a2t{t0}")

                a2cs = {}

                def a2_chunk(n, half):
                    def go():
                        if n not in a2cs:
                            a2cs[n] = eps.tile([128, 512], F32, tag="a2c",
                                               name=f"a2c{t0}_{n}")
                        a2c = a2cs[n]
                        for k in range(half * 8, half * 8 + 8):
                            if k < KC:
                                lhs = h2p[:, k, 0:nsteps, :].rearrange(
                                    "p s b -> p (s b)")
                            else:
                                lhs = ctxp[:, k - KC, 0:nsteps, :].rearrange(
                                    "p s b -> p (s b)")
                            nc.tensor.matmul(
                                a2c[0:M, :], lhs,
                                wa_sb[:, k, n * 512:(n + 1) * 512],
                                start=(k == 0), stop=(k == 15),
                            )
                        if half == 1:
                            nc.scalar.copy(a2sb[0:M, n * 512:(n + 1) * 512],
                                           a2c[0:M, :])
                    return go

                def tr_chunk():
                    for c2 in range(KC):
                        tpe = tps.tile([128, 128], BF16, tag="tp",
                                       name=f"a2tp{t0}_{c2}")
                        nc.tensor.transpose(
                            tpe[:, 0:M], a2sb[0:M, c2 * 128:(c2 + 1) * 128],
                            identb[0:M, 0:M])
                        nc.scalar.copy(a2t[:, c2, 0:M], tpe[:, 0:M])

                def fc_chunk(g):
                    def go():
                        lg = fps.tile([128, GS], F32, tag="lg",
                                      name=f"lg{t0}_{g}")
                        for k in range(KC):
                            nc.tensor.matmul(
                                lg[0:M, :], a2t[:, k, 0:M],
                                fcw_sb[:, k, g * GS:(g + 1) * GS],
                                start=(k == 0), stop=(k == KC - 1),
                            )
                        lgs = ework.tile([128, GS], F32, tag="lgs",
                                         name=f"lgs{t0}_{g}")
                        nc.vector.scalar_tensor_tensor(
                            lgs[0:M, :], lg[0:M, :], 1.0,
                            fcbR[0:M, g * GS:(g + 1) * GS], MULT, ADD)
                        for i in range(nsteps):
                            nc.gpsimd.dma_start(
                                out_d[:, t0 + i, g * GS:(g + 1) * GS],
                                lgs[i * 64:(i + 1) * 64, :],
                            )
                    return go

                for n in range(2):
                    for half in range(2):
                        epiq.append((1.8, a2_chunk(n, half)))
                epiq.append((2.6, tr_chunk))
                for g in range(8):
                    epiq.append((1.8, fc_chunk(g)))

            # ================= decode loop =================
            h2p_cur = ctxp_cur = None
            for t in range(T_DEC):
                xpt = work.tile([B, 512], BF16, tag="xp")
                nc.sync.dma_start(xpt[:], xp_d[t])

                # z = [H*,ctx] @ wz in two PSUM groups: the H*-half is
                # double-buffered so it can run during the prior AG2 flight
                zqA = zps.tile([B, 512], F32, tag="zqA")
                for k in range(KC):
                    nc.tensor.matmul(
                        zqA[:], h2T[:, k], wz[:, k],
                        start=(k == 0), stop=(k == KC - 1),
                    )
                zqB = zpsB.tile([B, 512], F32, tag="zqB")
                for k in range(KC, 16):
                    nc.tensor.matmul(
                        zqB[:], ctxTz[:, k - KC].rearrange("p q j -> p (q j)"),
                        wz[:, k],
                        start=(k == KC), stop=(k == 15),
                    )
                pump(2.0)

                # gates: layout [i|f|o|g] x128; H* = 2h, D = 2c
                z2 = gat.tile([B, 512], F32, tag="z2")
                nc.vector.scalar_tensor_tensor(z2[:], zqA[:], 1.0, xpt[:], MULT, ADD)
                nc.vector.scalar_tensor_tensor(z2[:], zqB[:], 1.0, z2[:], MULT, ADD)
                tio = gat.tile([B, 384], F32, tag="tio")
                nc.scalar.activation(tio[:], z2[:, 0:384], TANH, scale=0.5)
                tg = gat.tile([B, 128], F32, tag="tg")
                nc.scalar.activation(tg[:], z2[:, 384:512], TANH)
                av = gat.tile([B, 128], F32, tag="av")
                nc.vector.scalar_tensor_tensor(av[:], tio[:, 128:256], 1.0, cst[:], ADD, MULT)
                bv = gat.tile([B, 128], F32, tag="bv")
                nc.vector.scalar_tensor_tensor(bv[:], tio[:, 0:128], 1.0, tg[:], ADD, MULT)
                nc.vector.scalar_tensor_tensor(cst[:], av[:], 0.5, bv[:], MULT, ADD)
                tc2 = gat.tile([B, 128], F32, tag="tc2")
                nc.scalar.activation(tc2[:], cst[:], TANH, scale=0.5)
                h2s = gat.tile([B, 128], BF16, tag="h2s")
                nc.vector.scalar_tensor_tensor(h2s[:], tio[:, 256:384], 1.0, tc2[:], ADD, MULT)

                # stage own H* slice [64, 128]; transpose happens in the
                # gather DMA (xbar)
                nc.sync.dma_start(ag1i[t][:], h2s[:])
                nc.gpsimd.collective_compute(
                    "AllGather", mybir.AluOpType.bypass,
                    replica_groups=rg,
                    ins=[ag1i[t][:]], outs=[ag1o[t][:]],
                )
                pump(4.0)
                nc.scalar.dma_start(
                    h2T[:].rearrange("p q b -> p (q b)"), ag1o[t][:],
                    transpose=True,
                )

                # ---- scores for own 8 samples via cross + diag-select ----
                crossP = aps.tile([64, 512], F32, tag="crossP")
                for c in range(KC):
                    nc.tensor.matmul(
                        crossP[:], h2T[:, c],
                        keysK[:, c].rearrange("p j t -> p (j t)"),
                        start=(c == 0), stop=(c == KC - 1),
                    )
                ecr = gat.tile([64, 512], F32, tag="ecr")
                nc.scalar.activation(ecr[:], crossP[:], EXP)
                combo = aps.tile([128, KC * BL + 3 * BL], F32, tag="combo")
                ctxPS = combo[:, 0:KC * BL].rearrange("p (c j) -> p c j", c=KC)
                smx = combo[0:64, KC * BL:]
                for j in range(BL):
                    nc.tensor.matmul(
                        smx[:, j:j + 1], ecr[:, j * 64:(j + 1) * 64],
                        isel[:, j:j + 1], start=True, stop=True,
                    )
                e8 = gat.tile([64, BL], F32, tag="e8")
                nc.vector.tensor_copy(e8[:], smx[:, 0:BL])
                nc.tensor.matmul(smx[0:1, BL:2 * BL], ones64[:], e8[:],
                                 start=True, stop=True)
                r18 = gat.tile([1, BL], F32, tag="r18")
                nc.vector.reciprocal(r18[:], smx[0:1, BL:2 * BL])
                nc.tensor.matmul(smx[:, 2 * BL:3 * BL], onesr[:], r18[:],
                                 start=True, stop=True)
                a8 = gat.tile([64, BL], F16, tag="a8")
                nc.vector.tensor_mul(a8[:], e8[:], smx[:, 2 * BL:3 * BL])
                # scatter: even own-samples -> upper half, odd -> lower half
                nc.vector.tensor_copy(
                    alignZ[0:64, :].rearrange("p (pr two) -> p pr two", two=2)[:, :, 0],
                    a8[:].rearrange("p (pr two) -> p pr two", two=2)[:, :, 0],
                )
                nc.vector.tensor_copy(
                    alignZ[64:128, :].rearrange("p (pr two) -> p pr two", two=2)[:, :, 1],
                    a8[:].rearrange("p (pr two) -> p pr two", two=2)[:, :, 1],
                )

                # ---- ctx (own batches, pair-packed block-diag) ----
                for pr in range(NPAIR):
                    for c in range(KC):
                        nc.tensor.matmul(
                            ctxPS[:, c, 2 * pr:2 * pr + 2],
                            memPK[:, pr, c, :],
                            alignZ[:, 2 * pr:2 * pr + 2],
                            start=True, stop=True,
                        )
                ctxo = work.tile([128, KC, BL], BF16, tag="ctxo")
                nc.scalar.copy(ctxo[:], ctxPS[:])
                nc.sync.dma_start(ag2i[t][:], ctxo[:].rearrange("p c j -> p (c j)"))
                nc.gpsimd.collective_compute(
                    "AllGather", mybir.AluOpType.bypass,
                    replica_groups=rg,
                    ins=[ag2i[t][:]], outs=[ag2o[t][:]],
                )
                pump(10.0)
                nc.scalar.dma_start(
                    ctxT[:],
                    ag2o[t].rearrange("(q p) (c j) -> p q c j", q=NC, c=KC),
                )
                nc.vector.tensor_copy(
                    ctxTz[:], ctxT[:].rearrange("p q c j -> p c q j"))

                # ---- pair history (SBUF) + epilogue enqueue ----
                if t % 2 == 0:
                    h2p_cur = hist.tile([128, KC, 2, 64], BF16, tag="h2p",
                                        name=f"h2p{t}")
                    ctxp_cur = hist.tile([128, KC, 2, 64], BF16, tag="ctxp",
                                         name=f"ctxp{t}")
                slot = t % 2
                nc.vector.tensor_copy(h2p_cur[:, :, slot, :], h2T[:])
                nc.vector.tensor_copy(
                    ctxp_cur[:, :, slot, :],
                    ctxTz[:].rearrange("p c q j -> p c (q j)"))
                if t % 2 == 1:
                    make_pair_chunks(t - 1, 2, h2p_cur, ctxp_cur)
                elif t == T_DEC - 1:
                    make_pair_chunks(t, 1, h2p_cur, ctxp_cur)

            # drain remaining epilogue work
            pump(1e9)

    nc.finalize()
    return nc


def _prep_inputs(inputs):
    bfnp = mybir.dt.np(BF16)
    f32 = lambda x: np.asarray(x, dtype=np.float32)
    tokens = np.asarray(inputs["tokens"])
    memory = f32(inputs["memory"])
    enc_h = f32(inputs["enc_h"])
    enc_c = f32(inputs["enc_c"])
    emb = f32(inputs["emb"])
    Wm = f32(inputs["Wm"])
    Wa = f32(inputs["Wa"])
    lstm_k = f32(inputs["lstm_k"])
    lstm_r = f32(inputs["lstm_r"])
    lstm_b = f32(inputs["lstm_b"])
    fc_w = f32(inputs["fc_w"])
    fc_b = f32(inputs["fc_b"])

    Wk_x = lstm_k[:E]
    Wk_a = lstm_k[E:]
    Rp = Wa[:H] @ Wk_a + lstm_r
    Cp = Wa[H:] @ Wk_a
    wz_full = np.concatenate([0.5 * Rp, Cp], axis=0)       # [2048, 4096]
    xs = emb[tokens]                                        # [B, T, E]
    xpb = (xs @ Wk_x + lstm_b).transpose(1, 0, 2).copy()    # [T, B, 4096]
    xpb[0] += enc_h @ lstm_r                                # t=0 folding fix
    keys = memory @ (0.5 * Wm)                              # [B, T_in, H]
    wa_eff = np.concatenate([0.5 * Wa[:H], Wa[H:]], axis=0)

    common = dict(
        idf=np.eye(64, dtype=np.float32),
        idb=np.eye(128, dtype=np.float32).astype(bfnp),
        wa=np.ascontiguousarray(wa_eff.reshape(16, 128, H)).astype(bfnp),
    )
    maps = []
    for r in range(NC):
        cols = np.concatenate([
            np.arange(r * 128, r * 128 + 128),
            H + np.arange(r * 128, r * 128 + 128),
            3 * H + np.arange(r * 128, r * 128 + 128),
            2 * H + np.arange(r * 128, r * 128 + 128),
        ])
        wz_r = np.ascontiguousarray(wz_full[:, cols]).reshape(16, 128, 512)
        xp_r = np.ascontiguousarray(xpb[:, :, cols])
        own = slice(r * BL, (r + 1) * BL)
        kk = keys[own].transpose(2, 0, 1)                   # [H, 8, T_in]
        keysK_r = np.ascontiguousarray(
            kk.reshape(KC, 128, BL, T_IN).transpose(1, 0, 2, 3))
        m = memory[own]                                     # [8, T_in, H]
        memPK_r = np.zeros((128, NPAIR, KC, 128), np.float16)
        for j in range(BL):
            memPK_r[(j % 2) * 64:(j % 2) * 64 + 64, j // 2] = (
                m[j].reshape(T_IN, KC, 128))
        isel_r = np.zeros((B, BL), np.float32)
        isel_r[r * BL + np.arange(BL), np.arange(BL)] = 1.0
        maps.append(dict(
            common,
            wz=wz_r.astype(bfnp),
            xp=xp_r.astype(bfnp),
            c0=np.ascontiguousarray(2.0 * enc_c[:, r * 128:(r + 1) * 128]),
            keysk=keysK_r.astype(bfnp),
            mempk=memPK_r,
            isel=isel_r,
            fcw=np.ascontiguousarray(
                fc_w[:, r * VS:(r + 1) * VS]).reshape(KC, 128, VS).astype(bfnp),
            fcbr=np.ascontiguousarray(
                np.broadcast_to(fc_b[r * VS:(r + 1) * VS], (128, VS)), np.float32),
        ))
    return maps


def kernel(**inputs):
    if "nc" not in _CACHE:
        _CACHE["nc"] = _build()
    nc = _CACHE["nc"]
    maps = _prep_inputs(inputs)
    res = run_bass_kernel_spmd(nc, maps, list(range(NC)))
    global LAST_RESULT
    LAST_RESULT = res
    out = np.concatenate([res.results[r]["out"] for r in range(NC)], axis=2)
    return out


LAST_RESULT = None


# revision 21
# speedup vs baseline: 1.1480x; 1.1480x over previous
"""Trainium2 Bass kernel for the Luong-attention LSTM decoder (nn_Decoder).

8-core strategy (v2, sharded recurrence + interleaved epilogue):
- Host folds Wa into the recurrence: z = [H*, ctx] @ wz + xp with
  wz = [0.5*(Wa_top@Wk_a + lstm_r); Wa_bot@Wk_a], xp precomputed (+bias,
  t=0 correction). State H* = 2*h2 (tanh-only gates, no ACT table
  switches); consumers pre-scaled by 0.5 on host.
- z matmul + gates column-sharded 8-way: core r computes gate columns
  [i_r|f_r|o_r|g_r] (128 each), keeps its c-state slice. AllGather #1
  reassembles transposed H*; AllGather #2 reassembles ctx. Single-DMA
  gathers.
- Attention batch-sharded (8 samples/core): cross-scores via 8 matmuls +
  matmul diagonal-select (SPMD-uniform via per-core one-hot isel).
- Epilogue (attn2 = [H*,ctx]@wa_eff, logits vocab-sharded 4000/core) is
  chunked and pumped into the AG-wait windows of later steps, keeping the
  PE HAM-warm and removing the serial tail. Pair history kept in SBUF.
"""
import sys

sys.path.insert(0, "/opt/trn_rl_repo")

from collections import deque

import numpy as np
import concourse.bass as bass
import concourse.tile as tile
from concourse import bacc, mybir
from concourse.bass_utils import run_bass_kernel_spmd

B, T_IN, T_DEC = 64, 64, 47
V, E, H = 32000, 512, 1024
NC = 8
BL = B // NC
VS = V // NC
KC = H // 128
NPAIR = BL // 2
GS = VS // 8
NPAIRS_T = (T_DEC + 1) // 2
F32 = mybir.dt.float32
F16 = mybir.dt.float16
BF16 = mybir.dt.bfloat16
TANH = mybir.ActivationFunctionType.Tanh
EXP = mybir.ActivationFunctionType.Exp
MULT = mybir.AluOpType.mult
ADD = mybir.AluOpType.add

_CACHE = {}


def _build():
    nc = bacc.Bacc(None, target_bir_lowering=False)

    wz_d = nc.dram_tensor("wz", [16, 128, 512], BF16, kind="ExternalInput")
    xp_d = nc.dram_tensor("xp", [T_DEC, B, 512], BF16, kind="ExternalInput")
    c0_d = nc.dram_tensor("c0", [B, 128], F32, kind="ExternalInput")
    keysk_d = nc.dram_tensor("keysk", [128, KC, BL, T_IN], BF16, kind="ExternalInput")
    mempk_d = nc.dram_tensor("mempk", [128, NPAIR, KC, 128], F16, kind="ExternalInput")
    isel_d = nc.dram_tensor("isel", [B, BL], F32, kind="ExternalInput")
    idf_d = nc.dram_tensor("idf", [64, 64], F32, kind="ExternalInput")
    wa_d = nc.dram_tensor("wa", [16, 128, H], BF16, kind="ExternalInput")
    fcw_d = nc.dram_tensor("fcw", [KC, 128, VS], BF16, kind="ExternalInput")
    fcbr_d = nc.dram_tensor("fcbr", [128, VS], F32, kind="ExternalInput")
    idb_d = nc.dram_tensor("idb", [128, 128], BF16, kind="ExternalInput")
    out_d = nc.dram_tensor("out", [B, T_DEC, VS], F32, kind="ExternalOutput")

    ag1i = [nc.dram_tensor(f"ag1i{t}", [64, 128], BF16) for t in range(T_DEC)]
    ag1o = [
        nc.dram_tensor(f"ag1o{t}", [NC * 64, 128], BF16, addr_space="Shared")
        for t in range(T_DEC)
    ]
    ag2i = [nc.dram_tensor(f"ag2i{t}", [128, 64], BF16) for t in range(T_DEC)]
    ag2o = [
        nc.dram_tensor(f"ag2o{t}", [NC * 128, 64], BF16, addr_space="Shared")
        for t in range(T_DEC)
    ]
    rg = [list(range(NC))]

    with tile.TileContext(nc) as tc:
        with (
            tc.tile_pool(name="one", bufs=1) as one,
            tc.tile_pool(name="work", bufs=2) as work,
            tc.tile_pool(name="gat", bufs=2) as gat,
            tc.tile_pool(name="hist", bufs=3) as hist,
            tc.tile_pool(name="ework", bufs=2) as ework,
            tc.tile_pool(name="zps", bufs=1, space="PSUM") as zps,
            tc.tile_pool(name="tps", bufs=2, space="PSUM") as tps,
            tc.tile_pool(name="aps", bufs=1, space="PSUM") as aps,
            tc.tile_pool(name="eps", bufs=2, space="PSUM") as eps,
            tc.tile_pool(name="fps", bufs=1, space="PSUM") as fps,
        ):
            # ---------------- resident tiles ----------------
            wz = one.tile([128, 16, 512], BF16)
            nc.sync.dma_start(wz[:], wz_d.rearrange("k p n -> p k n"))
            keysK = one.tile([128, KC, BL, T_IN], BF16)
            nc.sync.dma_start(keysK[:], keysk_d[:])
            memPK = one.tile([128, NPAIR, KC, 128], F16)
            nc.sync.dma_start(memPK[:], mempk_d[:])
            isel = one.tile([B, BL], F32, tag="isel")
            nc.sync.dma_start(isel[:], isel_d[:])
            idf = one.tile([64, 64], F32, tag="idf")
            nc.sync.dma_start(idf[:], idf_d[:])
            cst = one.tile([B, 128], F32, tag="cst")
            nc.sync.dma_start(cst[:], c0_d[:])
            wa_sb = one.tile([128, 16, H], BF16)
            nc.sync.dma_start(wa_sb[:], wa_d.rearrange("k p h -> p k h"))
            fcw_sb = one.tile([128, KC, VS], BF16)
            nc.sync.dma_start(fcw_sb[:], fcw_d.rearrange("k p v -> p k v"))
            identb = one.tile([128, 128], BF16)
            nc.sync.dma_start(identb[:], idb_d[:])
            fcbR = one.tile([128, VS], F32)
            nc.sync.dma_start(fcbR[:], fcbr_d[:])

            h2T = one.tile([128, KC, 64], BF16, tag="h2T")
            nc.vector.memset(h2T[:], 0.0)
            ctxT = one.tile([128, NC, KC, BL], BF16, tag="ctxT")
            nc.vector.memset(ctxT[:], 0.0)
            ctxTz = one.tile([128, KC, NC, BL], BF16, tag="ctxTz")
            nc.vector.memset(ctxTz[:], 0.0)
            alignZ = one.tile([128, BL], F16, tag="alignZ")
            nc.vector.memset(alignZ[:], 0.0)
            ones64 = one.tile([B, 1], F32, tag="ones64")
            nc.vector.memset(ones64[:], 1.0)
            onesr = one.tile([1, B], F32, tag="onesr")
            nc.vector.memset(onesr[:], 1.0)

            # ------------- epilogue chunk machinery -------------
            epiq = deque()

            def pump(budget_us):
                spent = 0.0
                while epiq and spent < budget_us:
                    cost, go = epiq.popleft()
                    go()
                    spent += cost

            def make_pair_chunks(t0, nsteps, h2p, ctxp):
                M = 64 * nsteps
                a2sb = ework.tile([128, H], BF16, tag="a2sb",
                                  name=f"a2sb{t0}")
                a2t = ework.tile([128, KC, 128], BF16, tag="a2t",
                                 name=f"a2t{t0}")

                a2cs = {}

                def a2_chunk(n, half):
                    def go():
                        if n not in a2cs:
                            a2cs[n] = eps.tile([128, 512], F32, tag="a2c",
                                               name=f"a2c{t0}_{n}")
                        a2c = a2cs[n]
                        for k in range(half * 8, half * 8 + 8):
                            if k < KC:
                                lhs = h2p[:, k, 0:nsteps, :].rearrange(
                                    "p s b -> p (s b)")
                            else:
                                lhs = ctxp[:, k - KC, 0:nsteps, :].rearrange(
                                    "p s b -> p (s b)")
                            nc.tensor.matmul(
                                a2c[0:M, :], lhs,
                                wa_sb[:, k, n * 512:(n + 1) * 512],
                                start=(k == 0), stop=(k == 15),
                            )
                        if half == 1:
                            nc.scalar.copy(a2sb[0:M, n * 512:(n + 1) * 512],
                                           a2c[0:M, :])
                    return go

                def tr_chunk():
                    for c2 in range(KC):
                        tpe = tps.tile([128, 128], BF16, tag="tp",
                                       name=f"a2tp{t0}_{c2}")
                        nc.tensor.transpose(
                            tpe[:, 0:M], a2sb[0:M, c2 * 128:(c2 + 1) * 128],
                            identb[0:M, 0:M])
                        nc.scalar.copy(a2t[:, c2, 0:M], tpe[:, 0:M])

                def fc_chunk(g):
                    def go():
                        lg = fps.tile([128, GS], F32, tag="lg",
                                      name=f"lg{t0}_{g}")
                        for k in range(KC):
                            nc.tensor.matmul(
                                lg[0:M, :], a2t[:, k, 0:M],
                                fcw_sb[:, k, g * GS:(g + 1) * GS],
                                start=(k == 0), stop=(k == KC - 1),
                            )
                        lgs = ework.tile([128, GS], F32, tag="lgs",
                                         name=f"lgs{t0}_{g}")
                        nc.vector.scalar_tensor_tensor(
                            lgs[0:M, :], lg[0:M, :], 1.0,
                            fcbR[0:M, g * GS:(g + 1) * GS], MULT, ADD)
                        for i in range(nsteps):
                            nc.gpsimd.dma_start(
                                out_d[:, t0 + i, g * GS:(g + 1) * GS],
                                lgs[i * 64:(i + 1) * 64, :],
                            )
                    return go

                for n in range(2):
                    for half in range(2):
                        epiq.append((1.8, a2_chunk(n, half)))
                epiq.append((2.6, tr_chunk))
                for g in range(8):
                    epiq.append((1.8, fc_chunk(g)))

            # ================= decode loop =================
            h2p_cur = ctxp_cur = None
            for t in range(T_DEC):
                xpt = work.tile([B, 512], BF16, tag="xp")
                nc.sync.dma_start(xpt[:], xp_d[t])

                # z = [H*,ctx] @ wz  (k 0..7: H* tiles; 8..15: ctx tiles)
                zq = zps.tile([B, 512], F32, tag="zq")
                for k in range(16):
                    if k < KC:
                        lhs = h2T[:, k]
                    else:
                        lhs = ctxTz[:, k - KC].rearrange("p q j -> p (q j)")
                    nc.tensor.matmul(
                        zq[:], lhs, wz[:, k],
                        start=(k == 0), stop=(k == 15),
                    )
                pump(2.0)

                # gates: layout [i|f|o|g] x128; H* = 2h, D = 2c
                z2 = gat.tile([B, 512], F32, tag="z2")
                nc.vector.scalar_tensor_tensor(z2[:], zq[:], 1.0, xpt[:], MULT, ADD)
                tio = gat.tile([B, 384], F32, tag="tio")
                nc.scalar.activation(tio[:], z2[:, 0:384], TANH, scale=0.5)
                tg = gat.tile([B, 128], F32, tag="tg")
                nc.scalar.activation(tg[:], z2[:, 384:512], TANH)
                av = gat.tile([B, 128], F32, tag="av")
                nc.vector.scalar_tensor_tensor(av[:], tio[:, 128:256], 1.0, cst[:], ADD, MULT)
                bv = gat.tile([B, 128], F32, tag="bv")
                nc.vector.scalar_tensor_tensor(bv[:], tio[:, 0:128], 1.0, tg[:], ADD, MULT)
                nc.vector.scalar_tensor_tensor(cst[:], av[:], 0.5, bv[:], MULT, ADD)
                tc2 = gat.tile([B, 128], F32, tag="tc2")
                nc.scalar.activation(tc2[:], cst[:], TANH, scale=0.5)
                h2s = gat.tile([B, 128], BF16, tag="h2s")
                nc.vector.scalar_tensor_tensor(h2s[:], tio[:, 256:384], 1.0, tc2[:], ADD, MULT)

                # stage own H* slice [64, 128]; transpose happens in the
                # gather DMA (xbar)
                nc.sync.dma_start(ag1i[t][:], h2s[:])
                nc.gpsimd.collective_compute(
                    "AllGather", mybir.AluOpType.bypass,
                    replica_groups=rg,
                    ins=[ag1i[t][:]], outs=[ag1o[t][:]],
                )
                pump(4.0)
                nc.scalar.dma_start(
                    h2T[:].rearrange("p q b -> p (q b)"), ag1o[t][:],
                    transpose=True,
                )

                # ---- scores for own 8 samples via cross + diag-select ----
                crossP = aps.tile([64, 512], F32, tag="crossP")
                for c in range(KC):
                    nc.tensor.matmul(
                        crossP[:], h2T[:, c],
                        keysK[:, c].rearrange("p j t -> p (j t)"),
                        start=(c == 0), stop=(c == KC - 1),
                    )
                ecr = gat.tile([64, 512], F32, tag="ecr")
                nc.scalar.activation(ecr[:], crossP[:], EXP)
                combo = aps.tile([128, KC * BL + 3 * BL], F32, tag="combo")
                ctxPS = combo[:, 0:KC * BL].rearrange("p (c j) -> p c j", c=KC)
                smx = combo[0:64, KC * BL:]
                for j in range(BL):
                    nc.tensor.matmul(
                        smx[:, j:j + 1], ecr[:, j * 64:(j + 1) * 64],
                        isel[:, j:j + 1], start=True, stop=True,
                    )
                e8 = gat.tile([64, BL], F32, tag="e8")
                nc.vector.tensor_copy(e8[:], smx[:, 0:BL])
                nc.tensor.matmul(smx[0:1, BL:2 * BL], ones64[:], e8[:],
                                 start=True, stop=True)
                r18 = gat.tile([1, BL], F32, tag="r18")
                nc.vector.reciprocal(r18[:], smx[0:1, BL:2 * BL])
                nc.tensor.matmul(smx[:, 2 * BL:3 * BL], onesr[:], r18[:],
                                 start=True, stop=True)
                # normalize + scatter fused: even own-samples -> upper
                # half, odd -> lower half
                rbv = smx[:, 2 * BL:3 * BL]
                nc.vector.tensor_mul(
                    alignZ[0:64, :].rearrange("p (pr two) -> p pr two", two=2)[:, :, 0],
                    e8[:].rearrange("p (pr two) -> p pr two", two=2)[:, :, 0],
                    rbv.rearrange("p (pr two) -> p pr two", two=2)[:, :, 0],
                )
                nc.vector.tensor_mul(
                    alignZ[64:128, :].rearrange("p (pr two) -> p pr two", two=2)[:, :, 1],
                    e8[:].rearrange("p (pr two) -> p pr two", two=2)[:, :, 1],
                    rbv.rearrange("p (pr two) -> p pr two", two=2)[:, :, 1],
                )

                # ---- ctx (own batches, pair-packed block-diag) ----
                for pr in range(NPAIR):
                    for c in range(KC):
                        nc.tensor.matmul(
                            ctxPS[:, c, 2 * pr:2 * pr + 2],
                            memPK[:, pr, c, :],
                            alignZ[:, 2 * pr:2 * pr + 2],
                            start=True, stop=True,
                        )
                ctxo = work.tile([128, KC, BL], BF16, tag="ctxo")
                nc.scalar.copy(ctxo[:], ctxPS[:])
                nc.sync.dma_start(ag2i[t][:], ctxo[:].rearrange("p c j -> p (c j)"))
                nc.gpsimd.collective_compute(
                    "AllGather", mybir.AluOpType.bypass,
                    replica_groups=rg,
                    ins=[ag2i[t][:]], outs=[ag2o[t][:]],
                )
                pump(10.0)
                nc.scalar.dma_start(
                    ctxT[:],
                    ag2o[t].rearrange("(q p) (c j) -> p q c j", q=NC, c=KC),
                )
                nc.vector.tensor_copy(
                    ctxTz[:], ctxT[:].rearrange("p q c j -> p c q j"))

                # ---- pair history (SBUF) + epilogue enqueue ----
                if t % 2 == 0:
                    h2p_cur = hist.tile([128, KC, 2, 64], BF16, tag="h2p",
                                        name=f"h2p{t}")
                    ctxp_cur = hist.tile([128, KC, 2, 64], BF16, tag="ctxp",
                                         name=f"ctxp{t}")
                slot = t % 2
                nc.vector.tensor_copy(h2p_cur[:, :, slot, :], h2T[:])
                nc.vector.tensor_copy(
                    ctxp_cur[:, :, slot, :],
                    ctxTz[:].rearrange("p c q j -> p c (q j)"))
                if t % 2 == 1:
                    make_pair_chunks(t - 1, 2, h2p_cur, ctxp_cur)
                elif t == T_DEC - 1:
                    make_pair_chunks(t, 1, h2p_cur, ctxp_cur)

            # drain remaining epilogue work
            pump(1e9)

    nc.finalize()
    return nc


def _prep_inputs(inputs):
    bfnp = mybir.dt.np(BF16)
    f32 = lambda x: np.asarray(x, dtype=np.float32)
    tokens = np.asarray(inputs["tokens"])
    memory = f32(inputs["memory"])
    enc_h = f32(inputs["enc_h"])
    enc_c = f32(inputs["enc_c"])
    emb = f32(inputs["emb"])
    Wm = f32(inputs["Wm"])
    Wa = f32(inputs["Wa"])
    lstm_k = f32(inputs["lstm_k"])
    lstm_r = f32(inputs["lstm_r"])
    lstm_b = f32(inputs["lstm_b"])
    fc_w = f32(inputs["fc_w"])
    fc_b = f32(inputs["fc_b"])

    Wk_x = lstm_k[:E]
    Wk_a = lstm_k[E:]
    Rp = Wa[:H] @ Wk_a + lstm_r
    Cp = Wa[H:] @ Wk_a
    wz_full = np.concatenate([0.5 * Rp, Cp], axis=0)       # [2048, 4096]
    xs = emb[tokens]                                        # [B, T, E]
    xpb = (xs @ Wk_x + lstm_b).transpose(1, 0, 2).copy()    # [T, B, 4096]
    xpb[0] += enc_h @ lstm_r                                # t=0 folding fix
    keys = memory @ (0.5 * Wm)                              # [B, T_in, H]
    wa_eff = np.concatenate([0.5 * Wa[:H], Wa[H:]], axis=0)

    common = dict(
        idf=np.eye(64, dtype=np.float32),
        idb=np.eye(128, dtype=np.float32).astype(bfnp),
        wa=np.ascontiguousarray(wa_eff.reshape(16, 128, H)).astype(bfnp),
    )
    maps = []
    for r in range(NC):
        cols = np.concatenate([
            np.arange(r * 128, r * 128 + 128),
            H + np.arange(r * 128, r * 128 + 128),
            3 * H + np.arange(r * 128, r * 128 + 128),
            2 * H + np.arange(r * 128, r * 128 + 128),
        ])
        wz_r = np.ascontiguousarray(wz_full[:, cols]).reshape(16, 128, 512)
        xp_r = np.ascontiguousarray(xpb[:, :, cols])
        own = slice(r * BL, (r + 1) * BL)
        kk = keys[own].transpose(2, 0, 1)                   # [H, 8, T_in]
        keysK_r = np.ascontiguousarray(
            kk.reshape(KC, 128, BL, T_IN).transpose(1, 0, 2, 3))
        m = memory[own]                                     # [8, T_in, H]
        memPK_r = np.zeros((128, NPAIR, KC, 128), np.float16)
        for j in range(BL):
            memPK_r[(j % 2) * 64:(j % 2) * 64 + 64, j // 2] = (
                m[j].reshape(T_IN, KC, 128))
        isel_r = np.zeros((B, BL), np.float32)
        isel_r[r * BL + np.arange(BL), np.arange(BL)] = 1.0
        maps.append(dict(
            common,
            wz=wz_r.astype(bfnp),
            xp=xp_r.astype(bfnp),
            c0=np.ascontiguousarray(2.0 * enc_c[:, r * 128:(r + 1) * 128]),
            keysk=keysK_r.astype(bfnp),
            mempk=memPK_r,
            isel=isel_r,
            fcw=np.ascontiguousarray(
                fc_w[:, r * VS:(r + 1) * VS]).reshape(KC, 128, VS).astype(bfnp),
            fcbr=np.ascontiguousarray(
                np.broadcast_to(fc_b[r * VS:(r + 1) * VS], (128, VS)), np.float32),
        ))
    return maps


def kernel(**inputs):
    if "nc" not in _CACHE:
        _CACHE["nc"] = _build()
    nc = _CACHE["nc"]
    maps = _prep_inputs(inputs)
    res = run_bass_kernel_spmd(nc, maps, list(range(NC)))
    global LAST_RESULT
    LAST_RESULT = res
    out = np.concatenate([res.results[r]["out"] for r in range(NC)], axis=2)
    return out


LAST_RESULT = None
